# revision 26
# baseline (speedup 1.0000x reference)
"""Trainium2 Bass kernel for nn_AttentionModule (batch-softmax attention + MLP).

Sharding: batch-parallel over the 8 NeuronCores (8 of 64 batches per core).
The softmax over dim=0 (batch) needs the cross-core sum of exp(scores); that
is a single f32 AllReduce of Z[h, n, m] per head (1 MB each), pipelined
against the next head's compute.  Everything else is local.

Per-core dataflow (b = local batch 0..7, h = head 0..3, chunks of 128):
  XT[e, n]      = x[b]^T                  (PE transpose)
  {Q,K,V}pre    = x @ W + bias            (grouped matmul, rank-1 bias matmul)
  {Q,K,V}       = LN(pre)                 (bn_stats + ACT apply, bf16)
  QT, KT        = Q^T, K^T                (PE transpose)
  E^T[m,n]      = exp((K Q^T)/11)         (scores^T matmul + ACT exp)
  Z             = sum_b E  --AllReduce--> Zg ; R = 1/Zg
  A^T[e,n]      = V^T-chunks @ (E^T * R)  (PSUM accumulation over m chunks)
  h1^T[d1,n]    = relu(W1^T A_cat^T + b1) (per-partition bias)
  mlp[n,d2]     = relu(h1 W2 + b2)        (rank-1 bias matmul)
  out           = LN(x + mlp) * gamma + beta
"""

import sys

for _p in ("/opt/trn_rl_repo", "/opt/pypackages"):
    if _p not in sys.path:
        sys.path.append(_p)

from contextlib import ExitStack

import numpy as np

import concourse.bass as bass
import concourse.tile as tile
from concourse import bacc, masks, mybir
from concourse.bass_utils import run_bass_kernel_spmd

B, N, E, H = 64, 512, 128, 4
NCORES = 8
BL = B // NCORES          # local batches per core
NCH = N // 128            # 128-row chunks per sequence
INV_SQRT_EMB = 1.0 / 11.0  # int(sqrt(128)) == 11, faithful to the reference
EPS = 1e-5

F32 = mybir.dt.float32
BF16 = mybir.dt.bfloat16
AF = mybir.ActivationFunctionType
ALU = mybir.AluOpType

EXP_BUFS = 2   # double-buffer exp tiles so head h+1 overlaps AllReduce(h)


def _bcast_ap(ap1d, parts):
    """View a 1-D DRAM AP broadcast across `parts` partitions."""
    return bass.AP(tensor=ap1d.tensor, offset=ap1d.offset,
                   ap=[[0, parts]] + list(ap1d.ap))


def _ln_apply(nc, stat, src_ap, out_ap, eps_tile):
    """LayerNorm (no affine) over the free axis: out = (x - mu) * rsqrt(var+eps)."""
    st6 = stat.tile([128, 6], F32, tag="st6")
    nc.vector.bn_stats(st6[:], src_ap)
    mv = stat.tile([128, 2], F32, tag="mv")
    nc.vector.bn_aggr(mv[:], st6[:])
    rstd = stat.tile([128, 1], F32, tag="rstd")
    nc.scalar.activation(rstd[:], mv[:, 1:2], AF.Sqrt, bias=eps_tile[:])
    nc.vector.reciprocal(rstd[:], rstd[:])
    nc.vector.tensor_scalar(out_ap, src_ap, mv[:, 0:1], rstd[:],
                            op0=ALU.subtract, op1=ALU.mult)


def _build():
    nc = bacc.Bacc(None, target_bir_lowering=False, debug=False)

    x_ext = nc.declare_dram_parameter("x", [BL, N, E], F32, isOutput=False)
    wq_ext = nc.declare_dram_parameter("Wq", [H, E, 128], F32, isOutput=False)
    bq_ext = nc.declare_dram_parameter("bq", [H, 128], F32, isOutput=False)
    wk_ext = nc.declare_dram_parameter("Wk", [H, E, 128], F32, isOutput=False)
    bk_ext = nc.declare_dram_parameter("bk", [H, 128], F32, isOutput=False)
    wv_ext = nc.declare_dram_parameter("Wv", [H, E, E], F32, isOutput=False)
    bv_ext = nc.declare_dram_parameter("bv", [H, E], F32, isOutput=False)
    w1_ext = nc.declare_dram_parameter("W1", [H * E, E], F32, isOutput=False)
    b1_ext = nc.declare_dram_parameter("b1", [E], F32, isOutput=False)
    w2_ext = nc.declare_dram_parameter("W2", [E, E], F32, isOutput=False)
    b2_ext = nc.declare_dram_parameter("b2", [E], F32, isOutput=False)
    gamma_ext = nc.declare_dram_parameter("gamma", [E], F32, isOutput=False)
    beta_ext = nc.declare_dram_parameter("beta", [E], F32, isOutput=False)
    out_ext = nc.declare_dram_parameter("out", [BL, N, E], F32, isOutput=True)

    with tile.TileContext(nc) as tc:
        with ExitStack() as ctx:
            _emit(ctx, tc, x_ext, wq_ext, bq_ext, wk_ext, bk_ext, wv_ext,
                  bv_ext, w1_ext, b1_ext, w2_ext, b2_ext, gamma_ext, beta_ext,
                  out_ext)
    nc.compile()
    return nc


def _emit(ctx, tc, x_ext, wq_ext, bq_ext, wk_ext, bk_ext, wv_ext, bv_ext,
          w1_ext, b1_ext, w2_ext, b2_ext, gamma_ext, beta_ext, out_ext):
    nc = tc.nc

    persist = ctx.enter_context(tc.tile_pool(name="persist", bufs=1))
    wstage = ctx.enter_context(tc.tile_pool(name="wstage", bufs=2))
    exp_pool = ctx.enter_context(tc.tile_pool(name="exp", bufs=EXP_BUFS))
    vh_pool = ctx.enter_context(tc.tile_pool(name="vh", bufs=2))
    qkt = ctx.enter_context(tc.tile_pool(name="qkt", bufs=2))
    zpool = ctx.enter_context(tc.tile_pool(name="z", bufs=2))
    sc = ctx.enter_context(tc.tile_pool(name="sc", bufs=3))
    mlpp = ctx.enter_context(tc.tile_pool(name="mlpp", bufs=2))
    stat = ctx.enter_context(tc.tile_pool(name="stat", bufs=8))
    dram = ctx.enter_context(tc.tile_pool(name="dram", bufs=4, space="DRAM"))

    ps_proj = ctx.enter_context(tc.tile_pool(name="ps_proj", bufs=3, space="PSUM"))
    ps_sc = ctx.enter_context(tc.tile_pool(name="ps_sc", bufs=3, space="PSUM"))
    ps_acc = ctx.enter_context(tc.tile_pool(name="ps_acc", bufs=2, space="PSUM"))

    # ---- tiny dummy collective: pulls the one-time collective-init
    # barrier (~50us) into the load phase instead of before the first
    # real AllReduce ----
    dmy_in = dram.tile([128, 4], F32, tag="dmy_in")
    dmy_out = dram.tile([128, 4], F32, tag="dmy_out")
    zt = wstage.tile([128, 4], F32, tag="zt")
    nc.vector.memset(zt[:], 0.0)
    nc.sync.dma_start(dmy_in[:], zt[:])
    nc.gpsimd.collective_compute("AllReduce", ALU.add,
                                 replica_groups=[list(range(NCORES))],
                                 ins=[dmy_in.opt()], outs=[dmy_out.opt()])

    # ---- constants & weights ----
    ident = persist.tile([128, 128], BF16)
    masks.make_identity(nc, ident[:])
    ones1 = persist.tile([1, 128], BF16)
    nc.vector.memset(ones1[:], 1.0)
    epst = persist.tile([128, 1], F32)
    nc.vector.memset(epst[:], EPS)

    # QKV weights: per head, [e, 3, 128] with slots (q, k, v)
    wqkv = persist.tile([128, H, 3, 128], BF16)
    bqkv = persist.tile([1, H, 3, 128], BF16)
    for h in range(H):
        stg = wstage.tile([128, 3, 128], F32, tag="wstg")
        nc.sync.dma_start(stg[:, 0, :], wq_ext[h])
        nc.sync.dma_start(stg[:, 1, :], wk_ext[h])
        nc.sync.dma_start(stg[:, 2, :], wv_ext[h])
        nc.vector.tensor_copy(wqkv[:, h, :, :], stg[:])
        bstg = wstage.tile([1, 3, 128], F32, tag="bstg")
        nc.sync.dma_start(bstg[:, 0, :], bq_ext[h : h + 1, :])
        nc.sync.dma_start(bstg[:, 1, :], bk_ext[h : h + 1, :])
        nc.sync.dma_start(bstg[:, 2, :], bv_ext[h : h + 1, :])
        nc.vector.tensor_copy(bqkv[:, h, :, :], bstg[:])

    w1 = persist.tile([128, H, 128], BF16)
    for hh in range(H):
        stg = wstage.tile([128, 128], F32, tag="w1stg")
        nc.sync.dma_start(stg[:], w1_ext[hh * 128 : (hh + 1) * 128, :])
        nc.vector.tensor_copy(w1[:, hh, :], stg[:])
    w2 = persist.tile([128, 128], BF16)
    stg = wstage.tile([128, 128], F32, tag="w1stg")
    nc.sync.dma_start(stg[:], w2_ext[:])
    nc.vector.tensor_copy(w2[:], stg[:])

    b1col = persist.tile([128, 1], F32)
    nc.sync.dma_start(b1col[:], b1_ext[:].rearrange("(p o) -> p o", o=1))
    b2row_s = persist.tile([1, 128], F32)
    nc.sync.dma_start(b2row_s[:], b2_ext[:].rearrange("(o f) -> o f", o=1))
    b2row = persist.tile([1, 128], BF16)
    nc.vector.tensor_copy(b2row[:], b2row_s[:])

    gammab = persist.tile([128, 128], F32)
    nc.sync.dma_start(gammab[:], _bcast_ap(gamma_ext[:], 128))
    betab = persist.tile([128, 128], F32)
    nc.sync.dma_start(betab[:], _bcast_ap(beta_ext[:], 128))

    # ---- x: load + transpose -> XT[e, b, n] (bf16) ----
    xt = persist.tile([128, BL, N], BF16)
    for b in range(BL):
        xf = wstage.tile([128, NCH, 128], F32, tag="xload")
        nc.sync.dma_start(xf[:], x_ext[b].rearrange("(c p) e -> p c e", p=128))
        xb = wstage.tile([128, NCH, 128], BF16, tag="xcast")
        nc.gpsimd.tensor_copy(xb[:], xf[:])
        for c in range(NCH):
            nc.sync.dma_start(xt[:, b, c * 128 : (c + 1) * 128], xb[:, c, :],
                              transpose=True)

    # A^T[e, (b, h), n] accumulated across heads for the MLP
    at = persist.tile([128, BL, H, N], BF16)

    replica = [list(range(NCORES))]

    def _mlp_b(b):
        p1 = ps_acc.tile([128, N], F32, tag="acc")
        for hh in range(H):
            nc.tensor.matmul(p1[:], w1[:, hh, :], at[:, b, hh, :],
                             start=(hh == 0), stop=(hh == H - 1))
        h1t = mlpp.tile([128, N], BF16, tag="h1t")
        nc.scalar.activation(h1t[:], p1[:], AF.Relu, bias=b1col[:])
        xres = mlpp.tile([128, NCH, 128], F32, tag="xres")
        nc.sync.dma_start(xres[:], x_ext[b].rearrange("(c p) e -> p c e", p=128))
        ys = mlpp.tile([128, NCH, 128], F32, tag="ys")
        for c in range(NCH):
            p2 = ps_proj.tile([128, 3, 128], F32, tag="proj")
            nc.tensor.matmul(p2[:, 0, :], h1t[:, c * 128 : (c + 1) * 128],
                             w2[:], start=True, stop=False)
            nc.tensor.matmul(p2[:, 0, :], ones1[:], b2row[:],
                             start=False, stop=True)
            nc.scalar.activation(ys[:, c, :], p2[:, 0, :], AF.Relu)
        nc.vector.tensor_add(ys[:], ys[:], xres[:])
        # final LN: per-chunk stats, batched sqrt/recip, per-chunk apply
        mv4 = stat.tile([128, NCH, 2], F32, tag="mv4")
        for c in range(NCH):
            st6 = stat.tile([128, 6], F32, tag="st6")
            nc.vector.bn_stats(st6[:], ys[:, c, :])
            nc.vector.bn_aggr(mv4[:, c, :], st6[:])
        rstd4 = stat.tile([128, NCH], F32, tag="rstd4")
        nc.scalar.activation(rstd4[:], mv4[:, :, 1], AF.Sqrt, bias=epst[:])
        nc.vector.reciprocal(rstd4[:], rstd4[:])
        yo = mlpp.tile([128, NCH, 128], F32, tag="yo")
        for c in range(NCH):
            nc.vector.tensor_scalar(yo[:, c, :], ys[:, c, :], mv4[:, c, 0:1],
                                    rstd4[:, c : c + 1],
                                    op0=ALU.subtract, op1=ALU.mult)
        def _mid_bcast(ap):
            return bass.AP(tensor=ap.tensor, offset=ap.offset,
                           ap=[list(ap.ap[0]), [0, NCH], list(ap.ap[1])])
        nc.vector.tensor_mul(yo[:], yo[:], _mid_bcast(gammab[:]))
        nc.vector.tensor_add(yo[:], yo[:], _mid_bcast(betab[:]))
        nc.sync.dma_start(out_ext[b].rearrange("(c p) e -> p c e", p=128), yo[:])

    # ---- attention, head-major so the Z AllReduce pipelines across heads ----
    pending_pv = None
    for h in range(H):
        expd = exp_pool.tile([128, BL, NCH, N], BF16, tag="exph")
        vh = vh_pool.tile([128, BL, NCH, 128], BF16, tag="vh")
        for b in range(BL):
            qt = qkt.tile([128, N], BF16, tag="qt")
            kt = qkt.tile([128, N], BF16, tag="kt")
            for c in range(NCH):
                # grouped q|k|v projection for this (b, h, n-chunk)
                pp = ps_proj.tile([128, 3, 128], F32, tag="proj")
                nc.tensor.matmul(pp[:], xt[:, b, c * 128 : (c + 1) * 128],
                                 wqkv[:, h, :, :], start=True, stop=False)
                nc.tensor.matmul(pp[:], ones1[:], bqkv[:, h, :, :],
                                 start=False, stop=True)
                # per-slot LN stats, batched sqrt/recip/nmr over the 3 slots
                st6 = stat.tile([128, 3, 6], F32, tag="st6")
                mv = stat.tile([128, 3, 2], F32, tag="mv")
                for s in range(3):
                    nc.vector.bn_stats(st6[:, s, :], pp[:, s, :])
                    nc.vector.bn_aggr(mv[:, s, :], st6[:, s, :])
                rstd3 = stat.tile([128, 3], F32, tag="rstd3")
                nc.scalar.activation(rstd3[:], mv[:, :, 1], AF.Sqrt,
                                     bias=epst[:])
                nc.vector.reciprocal(rstd3[:], rstd3[:])
                nmr3 = stat.tile([128, 3], F32, tag="nmr3")
                nc.vector.scalar_tensor_tensor(nmr3[:], mv[:, :, 0], -1.0,
                                               rstd3[:], op0=ALU.mult,
                                               op1=ALU.mult)
                qn = sc.tile([128, 128], BF16, tag="qn")
                kn = sc.tile([128, 128], BF16, tag="kn")
                nc.scalar.activation(qn[:], pp[:, 0, :], AF.Identity,
                                     bias=nmr3[:, 0:1], scale=rstd3[:, 0:1])
                nc.scalar.activation(kn[:], pp[:, 1, :], AF.Identity,
                                     bias=nmr3[:, 1:2], scale=rstd3[:, 1:2])
                nc.vector.tensor_scalar(vh[:, b, c, :], pp[:, 2, :],
                                        mv[:, 2, 0:1], rstd3[:, 2:3],
                                        op0=ALU.subtract, op1=ALU.mult)
                nc.sync.dma_start(qt[:, c * 128 : (c + 1) * 128], qn[:],
                                  transpose=True)
                nc.sync.dma_start(kt[:, c * 128 : (c + 1) * 128], kn[:],
                                  transpose=True)
            for c in range(NCH):
                # scores^T[m_chunk, n] then exp
                ss = ps_sc.tile([128, N], F32, tag="sc")
                nc.tensor.matmul(ss[:], kt[:, c * 128 : (c + 1) * 128], qt[:],
                                 start=True, stop=True)
                nc.scalar.activation(expd[:, b, c, :], ss[:], AF.Exp,
                                     scale=INV_SQRT_EMB)

        # Z = sum_b exp via PE identity-accumulation -> one AllReduce per head
        zl = zpool.tile([128, NCH, N], F32, tag="zl")
        for c in range(NCH):
            zps = ps_acc.tile([128, N], F32, tag="acc")
            for b in range(BL):
                nc.tensor.matmul(zps[:], ident[:], expd[:, b, c, :],
                                 start=(b == 0), stop=(b == BL - 1))
            nc.scalar.activation(zl[:, c, :], zps[:], AF.Copy)
        zin = dram.tile([128, NCH, N], F32, tag="zin")
        nc.sync.dma_start(zin[:], zl[:])
        zout = dram.tile([128, NCH, N], F32, tag="zout")
        nc.gpsimd.collective_compute("AllReduce", ALU.add,
                                     replica_groups=replica,
                                     ins=[zin.opt()], outs=[zout.opt()])
        zg = zpool.tile([128, NCH, N], F32, tag="zg")
        nc.sync.dma_start(zg[:], zout[:])
        rr = zpool.tile([128, NCH, N], BF16, tag="rr")
        with nc.allow_low_precision(reason="softmax denom in bf16"):
            nc.vector.reciprocal(rr[:], zg[:])

        # defer P@V(h): emit it after head h+1's scores so the PE stream
        # never blocks on AR(h) (per-engine streams execute in emission
        # order; AR(h) completes while h+1's projections/scores run)
        def _pv(h=h, expd=expd, vh=vh, rr=rr):
            for b in range(BL):
                pa = ps_acc.tile([128, N], F32, tag="acc")
                for c in range(NCH):
                    pnorm = sc.tile([128, N], BF16, tag="pnorm")
                    nc.vector.tensor_mul(pnorm[:], expd[:, b, c, :],
                                         rr[:, c, :])
                    nc.tensor.matmul(pa[:], vh[:, b, c, :], pnorm[:],
                                     start=(c == 0), stop=(c == NCH - 1))
                nc.scalar.activation(at[:, b, h, :], pa[:], AF.Copy)
                if h == H - 1:
                    _mlp_b(b)

        if pending_pv is not None:
            pending_pv()
        pending_pv = _pv

    pending_pv()


_NC_CACHE = None


def kernel(**inputs):
    global _NC_CACHE
    if _NC_CACHE is None:
        _NC_CACHE = _build()
    nc = _NC_CACHE

    weights = {k: np.ascontiguousarray(np.asarray(v, dtype=np.float32))
               for k, v in inputs.items() if k != "x"}
    x = np.asarray(inputs["x"], dtype=np.float32)
    in_maps = []
    for c in range(NCORES):
        m = dict(weights)
        m["x"] = np.ascontiguousarray(x[c * BL : (c + 1) * BL])
        in_maps.append(m)

    res = run_bass_kernel_spmd(nc, in_maps, list(range(NCORES)))
    out = np.concatenate([res.results[c]["out"] for c in range(NCORES)], axis=0)
    return out.astype(np.float32)


if __name__ == "__main__":
    nc = _build()
    print("built ok:", len(nc.m.functions[0].instructions) if hasattr(nc.m.functions[0], "instructions") else "?")
